# revision 10
# baseline (speedup 1.0000x reference)
"""TRN2 Bass kernel for nn_DoubleGSOFTCrossAttnProcessor.

Strategy
--------
The GSOFT block-diagonal orthogonal transforms (Cayley maps of tiny [16,b,b]
parameter blocks) are linear, so they fold into the dense projection weights
on the host:

    q = q_scale * gsoft(gsoft(x, Pq_in) @ Wq.T, Pq_out)
      = x @ [BD(Q(Pq_in)) @ Wq.T @ BD(Q(Pq_out)) @ diag(q_scale)] = x @ Wq_eff

(same for k, v and the output projection; the bias is added on the host after
the device pass). The K/V paths only touch the tiny encoder states (77x768),
so K^T is computed on the host and shipped as a constant. The key structural
move: attnout_h = probs_h @ V_h has rank <= 77, so the output projection
collapses into per-head [77, 1280] matrices precomputed on the host,

    out = sum_h probs_h @ M_h,   M_h = V_h @ Wout_eff[head-h rows, :]

which removes the entire 1280x1280 output-projection pass (and the attnout
eviction traffic) from the device. The device kernel is then, per core (one
batch element per NeuronCore, no collectives):

  - B phase: Q^T = Wq_eff^T @ x^T per 512-seq tile; x and Wq ship bf16
    (halves their DMA; the induced ~0.4% q error is well inside the 2e-2
    gate), Q^T evicted f32r on ScalarE. Wq is chunked m-major into 10 tiles
    so the first matmul starts ~2us in; each tile's x is prefetched one tile
    ahead.
  - C phase per head: scores^T = K^T_h^T @ Q^T_h (f32r, keys on partitions);
    exp on ScalarE (no max-subtraction: scores are O(5), fp32 exp cannot
    overflow); denominator via partition_all_reduce on the otherwise-idle
    Pool engine; single-instruction approx reciprocal (DVE) of the [1,512]
    row; partition_broadcast (Pool) back to 77 rows; probs = exp * recip in
    place on DVE. Then 10 accumulation matmuls out^T[n] += M_h^T @ probs^T_h
    (K=77). out^T PSUM banks are DMA'd straight to DRAM; the host
    de-transposes. The next tile's B-phase matmul groups are interleaved
    into the attention pipeline's dependency gaps as PE fillers.

HEAD_PERM: head h's first 128 score/value features -> chunk h; its last 32
packed into chunks 8-9 at row 32*(h%4). Applied to Wq/Wk columns, Wv columns
and Wout rows on the host, which makes every matmul operand partition-aligned
(the 160-dim head size is otherwise hostile to the 128-partition PE
geometry).
"""


import numpy as np
import ml_dtypes
from contextlib import ExitStack

import concourse.bass as bass
import concourse.bass_isa as bass_isa
import concourse.tile as tile
from concourse import bacc, mybir
from concourse.bass_isa import ReduceOp

F32 = mybir.dt.float32
F32R = mybir.dt.float32r
BF16 = mybir.dt.bfloat16

HID, CROSS, NBLK, HEADS = 1280, 768, 16, 8
HEAD_DIM = HID // HEADS               # 160
ATTN_SCALE = HEAD_DIM ** -0.5
SEQ, SKEY = 4096, 77
SKP = 80                              # padded key count (even, f32r requirement)
SQ = 512                              # seq-tile size
NT = SEQ // SQ                        # 8 seq tiles
KH = HID // 128                       # 10 feature chunks
XH = KH * SQ // 2                     # xt half-tile free size (2560)
# out^T accumulation passes over the 10 feature chunks (PSUM budget: 4+2+2=8
# banks with B-phase psum and scores)
PASSES = [(0, 4), (4, 4), (8, 2)]


def _cayley(P):
    P = P.astype(np.float64)
    A = P - np.swapaxes(P, -1, -2)
    I = np.eye(P.shape[-1], dtype=np.float64)
    return np.linalg.solve(I[None] - A, np.broadcast_to(I, A.shape) + A)


def _fold(P_in, W, P_out, scale):
    """W_eff = BD(Q_in) @ W.T @ BD(Q_out) @ diag(scale); W is [out, in]."""
    Qi, Qo = _cayley(P_in), _cayley(P_out)
    WT = W.astype(np.float64).T
    g, b = Qi.shape[0], Qi.shape[1]
    T1 = np.einsum("gij,gjc->gic", Qi, WT.reshape(g, b, -1)).reshape(WT.shape)
    go, bo = Qo.shape[0], Qo.shape[1]
    T2 = np.einsum("rgi,gij->rgj", T1.reshape(-1, go, bo), Qo).reshape(WT.shape)
    return T2 * scale.astype(np.float64)[None, :]


def _head_perm():
    """head h's first 128 features -> chunk h; last 32 -> chunk 8/9 row 32*(h%4)."""
    perm = np.empty(HID, np.int64)
    for h in range(HEADS):
        perm[128 * h : 128 * h + 128] = np.arange(160 * h, 160 * h + 128)
        perm[1024 + 32 * h : 1024 + 32 * h + 32] = np.arange(
            160 * h + 128, 160 * h + 160)
    return perm


HEAD_PERM = _head_perm()


def _head_idx(h):
    return np.r_[128 * h : 128 * h + 128, 1024 + 32 * h : 1024 + 32 * h + 32]


def fold_weights(inputs):
    wq = _fold(inputs["Pq_in"], inputs["Wq"], inputs["Pq_out"], inputs["q_scale"])
    wk = _fold(inputs["Pk_in"], inputs["Wk"], inputs["Pk_out"], inputs["k_scale"])
    wv = _fold(inputs["Pv_in"], inputs["Wv"], inputs["Pv_out"], inputs["v_scale"])
    wo = _fold(inputs["Pout_in"], inputs["Wout"], inputs["Pout_out"],
               inputs["out_scale"])
    wq = wq[:, HEAD_PERM]
    wk = wk[:, HEAD_PERM]
    wv = wv[:, HEAD_PERM]
    wo = wo[HEAD_PERM, :]
    return wq, wk, wv, wo  # float64 [in, out]


def make_in_map(x_b, enc_b, wq, wk, wv, wo):
    # x^T tiles, bf16: [NT, 2, 128, XH]
    xt = (x_b.T.reshape(KH, 128, NT, SQ).transpose(2, 1, 0, 3)
          .reshape(NT, 128, 2, XH).transpose(0, 2, 1, 3))
    xt = np.ascontiguousarray(xt.astype(ml_dtypes.bfloat16))
    # Wq m-major chunks, bf16: wqm[m][:, k*128:(k+1)*128] = wq[k-chunk, m-chunk]
    wqm = (wq.reshape(KH, 128, KH, 128).transpose(2, 1, 0, 3)
           .reshape(KH, 128, HID))
    wqm = np.ascontiguousarray(wqm.astype(ml_dtypes.bfloat16))
    # Host K^T (tiny): K = encp @ wk  [80, 1280]
    encp = np.zeros((SKP, CROSS), np.float64)
    encp[:SKEY] = enc_b
    K = encp @ wk                                    # [80, 1280]
    V = encp @ wv                                    # [80, 1280]
    kt = np.ascontiguousarray(
        K.T.reshape(KH, 128, SKP).transpose(1, 0, 2).reshape(128, KH * SKP)
    ).astype(np.float32)
    # Per-head output factors M_h = V_h @ Wout_h  [77, 1280], packed
    # [128, 8*1280] (keys on partitions, zeros beyond row 76)
    M = np.zeros((128, HEADS * HID), np.float32)
    for h in range(HEADS):
        idx = _head_idx(h)
        M[:SKEY, h * HID : (h + 1) * HID] = (V[:SKEY, idx] @ wo[idx, :])
    return {"xt": xt, "wqm": wqm, "kt": kt, "m": M}


def _head_pieces(h):
    return [(h, 0, 128), (8 + h // 4, 32 * (h % 4), 32)]


def build_nc(loop_reps=1):
    nc = bacc.Bacc("TRN2", target_bir_lowering=False, debug=False)
    xt_d = nc.dram_tensor("xt", [NT, 2, 128, XH], BF16, kind="ExternalInput").ap()
    wqm_d = nc.dram_tensor("wqm", [KH, 128, HID], BF16, kind="ExternalInput").ap()
    kt_d = nc.dram_tensor("kt", [128, KH * SKP], F32R, kind="ExternalInput").ap()
    m_d = nc.dram_tensor("m", [128, HEADS * HID], F32R, kind="ExternalInput").ap()
    out_d = nc.dram_tensor("out", [NT, KH, 128, SQ], F32,
                           kind="ExternalOutput").ap()

    with tile.TileContext(nc) as tc:
        with ExitStack() as ctx:
            ctx.enter_context(nc.allow_low_precision(
                "bf16/f32r matmul inputs; accumulation stays f32 in PSUM"))
            const = ctx.enter_context(tc.tile_pool(name="const", bufs=1))
            wqm_t = []
            for m in range(KH):
                wqm_t.append(const.tile([128, HID], BF16, name=f"wqm{m}"))
            nc.sync.dma_start(wqm_t[0][:], wqm_d[0])
            kt_t = const.tile([128, KH * SKP], F32R, name="kt_t")
            m_t = const.tile([128, HEADS * HID], F32R, name="m_t")

            def dma_const_early():
                nc.sync.dma_start(wqm_t[1][:], wqm_d[1])
                nc.sync.dma_start(kt_t[:], kt_d)

            def dma_const_rest():
                nc.sync.dma_start(m_t[:], m_d)
                for m in range(2, KH):
                    nc.sync.dma_start(wqm_t[m][:], wqm_d[m])

            xt_pool = ctx.enter_context(tc.tile_pool(name="xt", bufs=4))
            qt_pool = ctx.enter_context(tc.tile_pool(name="qt", bufs=2))
            psum_mm = ctx.enter_context(
                tc.tile_pool(name="psum_mm", bufs=2, space="PSUM"))

            xt_tiles = {}

            def prefetch_xt(t):
                xh = []
                for hf in range(2):
                    xx = xt_pool.tile([128, XH], BF16, tag="xt", name=f"xt{t}_{hf}")
                    nc.sync.dma_start(xx[:], xt_d[t, hf])
                    xh.append(xx)
                xt_tiles[t] = xh

            if loop_reps > 1:
                dma_const_early()
                dma_const_rest()
                # hint_engines: the ~2000-inst body exceeds IRAM blocks, so
                # prefetch the back-edge target (else ~4us I$ miss/iteration
                # inflates the measured per-pass slope)
                ctx.enter_context(tc.For_i(
                    0, loop_reps, 1,
                    hint_engines=(mybir.EngineType.PE, mybir.EngineType.DVE,
                                  mybir.EngineType.Activation,
                                  mybir.EngineType.SP, mybir.EngineType.Pool)))
                prefetch_xt(0)
                prefetch_xt(1)
            else:
                prefetch_xt(0)
                dma_const_early()
                prefetch_xt(1)
                dma_const_rest()

            qt_tiles = {}

            def b_group_makers(t):
                """B-phase of tile t as closures: per m-group, two 5-matmul
                halves + ScalarE eviction; first call prefetches xt(t+1)."""
                qt_t = qt_pool.tile([128, KH * SQ], F32R, tag="qt", name=f"qt{t}")
                qt_tiles[t] = qt_t
                state = {}
                makers = []

                def mk(m, half):
                    def run():
                        if m == 0 and half == 0 and t + 1 < NT \
                                and t + 1 not in xt_tiles:
                            prefetch_xt(t + 1)
                        if m not in state:
                            state[m] = psum_mm.tile([128, SQ], F32, tag="mm",
                                                    name=f"pq{t}_{m}")
                        pq = state[m]
                        xh = xt_tiles[t]
                        for k in range(5 * half, 5 * half + 5):
                            nc.tensor.matmul(
                                pq[:],
                                wqm_t[m][:, k * 128 : (k + 1) * 128],
                                xh[k // 5][:, (k % 5) * SQ : (k % 5 + 1) * SQ],
                                start=(k == 0), stop=(k == KH - 1),
                            )
                        if half == 1:
                            nc.scalar.copy(qt_t[:, m * SQ : (m + 1) * SQ], pq[:])
                            if m == KH - 1:
                                xt_tiles.pop(t)
                    return run

                for m in range(KH):
                    makers.append(mk(m, 0))
                    makers.append(mk(m, 1))
                return makers

            exp_pool = ctx.enter_context(tc.tile_pool(name="exp", bufs=8))
            den_pool = ctx.enter_context(tc.tile_pool(name="den", bufs=2))
            rd_pool = ctx.enter_context(tc.tile_pool(name="rd", bufs=2))
            rdb_pool = ctx.enter_context(tc.tile_pool(name="rdb", bufs=2))
            out_pool = ctx.enter_context(tc.tile_pool(name="outsb", bufs=3))
            psum_at = ctx.enter_context(
                tc.tile_pool(name="psum_at", bufs=2, space="PSUM"))
            psum_out = ctx.enter_context(
                tc.tile_pool(name="psum_out", bufs=4, space="PSUM"))

            def phase_C(t, fillers):
                """Attention + rank-77 output accumulation; `fillers` (B-phase
                groups of t+1) emitted between dependent links as PE gap
                fillers."""
                qt_t = qt_tiles.pop(t)
                sc_tiles, exp_tiles, rds = {}, {}, {}
                po = {}

                def fill(n=1):
                    for _ in range(n):
                        if fillers:
                            fillers.pop(0)()

                def out_mms(n_lo, n_cnt, h):
                    for n in range(n_lo, n_lo + n_cnt):
                        if n not in po:
                            po[n] = psum_out.tile([128, SQ], F32, tag="po",
                                                  name=f"po{t}_{n}")
                        nc.tensor.matmul(
                            po[n][:],
                            m_t[0:SKEY, h * HID + n * 128 : h * HID + (n + 1) * 128],
                            exp_tiles[h][:],
                            start=(h == 0), stop=(h == HEADS - 1),
                        )
                        if h == HEADS - 1:
                            ob = out_pool.tile([128, SQ], F32, tag="ob",
                                               name=f"ob{t}_{n}")
                            nc.scalar.copy(ob[:], po.pop(n)[:])
                            nc.sync.dma_start(out_d[t, n], ob[:])

                def stage_sc(h):  # scoresT
                    sc = psum_at.tile([SKP, SQ], F32, tag="attn", name=f"sc{t}_{h}")
                    for i, (c, o, L) in enumerate(_head_pieces(h)):
                        nc.tensor.matmul(
                            sc[:],
                            kt_t[o : o + L, c * SKP : (c + 1) * SKP],
                            qt_t[o : o + L, c * SQ : (c + 1) * SQ],
                            start=(i == 0), stop=(i == 1),
                            tile_position=(o, 0),
                        )
                    sc_tiles[h] = sc

                def stage_mid(h):  # exp, den on Pool, 1/den on DVE
                    sc = sc_tiles.pop(h)
                    exp_h = exp_pool.tile([SKEY, SQ], F32R, tag="exp",
                                          name=f"ex{t}_{h}")
                    nc.scalar.activation(
                        exp_h[:], sc[0:SKEY, :],
                        mybir.ActivationFunctionType.Exp, scale=ATTN_SCALE,
                    )
                    exp_tiles[h] = exp_h
                    den = den_pool.tile([SKEY, SQ], F32, tag="den",
                                        name=f"dn{t}_{h}")
                    nc.gpsimd.partition_all_reduce(
                        den[:], exp_h[:], SKEY, ReduceOp.add)
                    rd = rd_pool.tile([1, SQ], F32, tag="rd", name=f"rd{t}_{h}")
                    nc.vector.reciprocal_approx_fast(rd[:], den[0:1, :])
                    rds[h] = rd

                def stage_out(h):  # broadcast 1/den, normalize, accumulate out
                    rdb = rdb_pool.tile([SKEY, SQ], F32, tag="rdb",
                                        name=f"rb{t}_{h}")
                    nc.gpsimd.partition_broadcast(rdb[:], rds.pop(h)[:], SKEY)
                    fill()
                    nc.vector.tensor_tensor(
                        exp_tiles[h][:], exp_tiles[h][:], rdb[:],
                        mybir.AluOpType.mult)
                    fill()
                    out_mms(PASSES[0][0], PASSES[0][1], h)

                for s in range(HEADS + 2):
                    if s < HEADS:
                        stage_sc(s)
                    fill()
                    if 0 <= s - 1 < HEADS:
                        stage_mid(s - 1)
                    fill()
                    if 0 <= s - 2 < HEADS:
                        stage_out(s - 2)
                for (n_lo, n_cnt) in PASSES[1:]:
                    for h in range(HEADS):
                        out_mms(n_lo, n_cnt, h)
                        fill()
                while fillers:
                    fillers.pop(0)()

            for run in b_group_makers(0):
                run()
            for t in range(NT):
                fillers = b_group_makers(t + 1) if t + 1 < NT else []
                phase_C(t, fillers)

    nc.finalize()
    return nc


from concourse.bass_utils import run_bass_kernel_spmd

_NC_CACHE = {}


def _get_nc(loop_reps=1):
    if loop_reps not in _NC_CACHE:
        _NC_CACHE[loop_reps] = build_nc(loop_reps)
    return _NC_CACHE[loop_reps]


def kernel(**inputs):
    inputs = {k: np.asarray(v) for k, v in inputs.items()}
    wq, wk, wv, wo = fold_weights(inputs)
    x = inputs["hidden_states"].astype(np.float32, copy=False)
    enc = inputs["encoder_hidden_states"].astype(np.float64, copy=False)
    B = x.shape[0]
    in_maps = [make_in_map(x[b], enc[b], wq, wk, wv, wo) for b in range(B)]
    nc = _get_nc()
    res = run_bass_kernel_spmd(nc, in_maps, list(range(B)))
    bout = inputs["bout"].astype(np.float32, copy=False)
    outs = []
    for b in range(B):
        ot = res.results[b]["out"]              # [NT, KH, 128, SQ]
        o = ot.transpose(0, 3, 1, 2).reshape(SEQ, HID)
        outs.append(o + bout[None, :])
    return np.stack(outs)


# revision 14
# speedup vs baseline: 1.1477x; 1.1477x over previous
"""TRN2 Bass kernel for nn_DoubleGSOFTCrossAttnProcessor.

Strategy
--------
The GSOFT block-diagonal orthogonal transforms (Cayley maps of tiny [16,b,b]
parameter blocks) are linear, so they fold into the dense projection weights
on the host:

    q = q_scale * gsoft(gsoft(x, Pq_in) @ Wq.T, Pq_out)
      = x @ [BD(Q(Pq_in)) @ Wq.T @ BD(Q(Pq_out)) @ diag(q_scale)] = x @ Wq_eff

(same for k, v and the output projection; the bias is added on the host after
the device pass). The K/V paths only touch the tiny encoder states (77x768),
so K^T is computed on the host and shipped as a constant. The key structural
move: attnout_h = probs_h @ V_h has rank <= 77, so the output projection
collapses into per-head [77, 1280] matrices precomputed on the host,

    out = sum_h probs_h @ M_h,   M_h = V_h @ Wout_eff[head-h rows, :]

which removes the entire 1280x1280 output-projection pass (and the attnout
eviction traffic) from the device. The device kernel is then, per core (one
batch element per NeuronCore, no collectives):

  - B phase: Q^T = Wq_eff^T @ x^T per 512-seq tile; x and Wq ship bf16
    (halves their DMA; the induced ~0.4% q error is well inside the 2e-2
    gate), Q^T evicted f32r on ScalarE. Wq is chunked m-major into 10 tiles
    so the first matmul starts ~2us in; each tile's x is prefetched one tile
    ahead.
  - C phase per head: scores^T = K^T_h^T @ Q^T_h (f32r, keys on partitions);
    exp on ScalarE (no max-subtraction: scores are O(5), fp32 exp cannot
    overflow); denominator via partition_all_reduce on the otherwise-idle
    Pool engine; single-instruction approx reciprocal (DVE) of the [1,512]
    row; partition_broadcast (Pool) back to 77 rows; probs = exp * recip in
    place on DVE. Then 10 accumulation matmuls out^T[n] += M_h^T @ probs^T_h
    (K=77). out^T PSUM banks are DMA'd straight to DRAM; the host
    de-transposes. The next tile's B-phase matmul groups are interleaved
    into the attention pipeline's dependency gaps as PE fillers.

HEAD_PERM: head h's first 128 score/value features -> chunk h; its last 32
packed into chunks 8-9 at row 32*(h%4). Applied to Wq/Wk columns, Wv columns
and Wout rows on the host, which makes every matmul operand partition-aligned
(the 160-dim head size is otherwise hostile to the 128-partition PE
geometry).
"""


import numpy as np
import ml_dtypes
from contextlib import ExitStack

import concourse.bass as bass
import concourse.bass_isa as bass_isa
import concourse.tile as tile
from concourse import bacc, mybir
from concourse.bass_isa import ReduceOp

F32 = mybir.dt.float32
F32R = mybir.dt.float32r
BF16 = mybir.dt.bfloat16

HID, CROSS, NBLK, HEADS = 1280, 768, 16, 8
HEAD_DIM = HID // HEADS               # 160
ATTN_SCALE = HEAD_DIM ** -0.5
SEQ, SKEY = 4096, 77
SKP = 80                              # padded key count (even, f32r requirement)
SQ = 512                              # seq-tile size
NT = SEQ // SQ                        # 8 seq tiles
KH = HID // 128                       # 10 feature chunks
XH = KH * SQ // 2                     # xt half-tile free size (2560)
# out^T accumulation passes over the 10 feature chunks (PSUM budget: 4+2+2=8
# banks with B-phase psum and scores)
PASSES = [(0, 4), (4, 4), (8, 2)]


def _cayley(P):
    P = P.astype(np.float64)
    A = P - np.swapaxes(P, -1, -2)
    I = np.eye(P.shape[-1], dtype=np.float64)
    return np.linalg.solve(I[None] - A, np.broadcast_to(I, A.shape) + A)


def _fold(P_in, W, P_out, scale):
    """W_eff = BD(Q_in) @ W.T @ BD(Q_out) @ diag(scale); W is [out, in]."""
    Qi, Qo = _cayley(P_in), _cayley(P_out)
    WT = W.astype(np.float64).T
    g, b = Qi.shape[0], Qi.shape[1]
    T1 = np.einsum("gij,gjc->gic", Qi, WT.reshape(g, b, -1)).reshape(WT.shape)
    go, bo = Qo.shape[0], Qo.shape[1]
    T2 = np.einsum("rgi,gij->rgj", T1.reshape(-1, go, bo), Qo).reshape(WT.shape)
    return T2 * scale.astype(np.float64)[None, :]


def _head_perm():
    """head h's first 128 features -> chunk h; last 32 -> chunk 8/9 row 32*(h%4)."""
    perm = np.empty(HID, np.int64)
    for h in range(HEADS):
        perm[128 * h : 128 * h + 128] = np.arange(160 * h, 160 * h + 128)
        perm[1024 + 32 * h : 1024 + 32 * h + 32] = np.arange(
            160 * h + 128, 160 * h + 160)
    return perm


HEAD_PERM = _head_perm()


def _head_idx(h):
    return np.r_[128 * h : 128 * h + 128, 1024 + 32 * h : 1024 + 32 * h + 32]


def fold_weights(inputs):
    wq = _fold(inputs["Pq_in"], inputs["Wq"], inputs["Pq_out"], inputs["q_scale"])
    wk = _fold(inputs["Pk_in"], inputs["Wk"], inputs["Pk_out"], inputs["k_scale"])
    wv = _fold(inputs["Pv_in"], inputs["Wv"], inputs["Pv_out"], inputs["v_scale"])
    wo = _fold(inputs["Pout_in"], inputs["Wout"], inputs["Pout_out"],
               inputs["out_scale"])
    wq = wq[:, HEAD_PERM]
    wk = wk[:, HEAD_PERM]
    wv = wv[:, HEAD_PERM]
    wo = wo[HEAD_PERM, :]
    return wq, wk, wv, wo  # float64 [in, out]


def make_in_map(x_b, enc_b, wq, wk, wv, wo):
    # x^T tiles, bf16: [NT, 2, 128, XH]
    xt = (x_b.T.reshape(KH, 128, NT, SQ).transpose(2, 1, 0, 3)
          .reshape(NT, 128, 2, XH).transpose(0, 2, 1, 3))
    xt = np.ascontiguousarray(xt.astype(ml_dtypes.bfloat16))
    # Wq m-major chunks, bf16: wqm[m][:, k*128:(k+1)*128] = wq[k-chunk, m-chunk]
    wqm = (wq.reshape(KH, 128, KH, 128).transpose(2, 1, 0, 3)
           .reshape(KH, 128, HID))
    wqm = np.ascontiguousarray(wqm.astype(ml_dtypes.bfloat16))
    # Host K^T (tiny): K = encp @ wk  [80, 1280]
    encp = np.zeros((SKP, CROSS), np.float64)
    encp[:SKEY] = enc_b
    K = encp @ wk                                    # [80, 1280]
    V = encp @ wv                                    # [80, 1280]
    # kt slots 0-7: head h's first-128 features; slots 8-15: head h's last-32
    # features placed at their in-chunk row offset, other rows zero -- keeps
    # every scores matmul at full 128-partition contraction (sub-96-row
    # matmuls run ~1.4x slower on HW)
    KT = K.T.astype(np.float32)                      # [1280, 80]
    kt = np.zeros((128, 2 * HEADS * 128), np.float32)
    for h in range(HEADS):
        kt[:, h * 128 : h * 128 + SKP] = KT[128 * h : 128 * (h + 1)]
        o = 32 * (h % 4)
        kt[o : o + 32, (8 + h) * 128 : (8 + h) * 128 + SKP] = \
            KT[1024 + 32 * h : 1024 + 32 * h + 32]
    # Per-head output factors M_h = V_h @ Wout_h  [77, 1280], packed
    # [128, 8*1280] (keys on partitions, zeros beyond row 76)
    M = np.zeros((128, HEADS * HID), np.float32)
    for h in range(HEADS):
        idx = _head_idx(h)
        M[:SKEY, h * HID : (h + 1) * HID] = (V[:SKEY, idx] @ wo[idx, :])
    return {"xt": xt, "wqm": wqm, "kt": kt, "m": M}


def _head_pieces(h):
    return [(h, 0, 128), (8 + h // 4, 32 * (h % 4), 32)]


def build_nc(loop_reps=1):
    nc = bacc.Bacc("TRN2", target_bir_lowering=False, debug=False)
    xt_d = nc.dram_tensor("xt", [NT, 2, 128, XH], BF16, kind="ExternalInput").ap()
    wqm_d = nc.dram_tensor("wqm", [KH, 128, HID], BF16, kind="ExternalInput").ap()
    kt_d = nc.dram_tensor("kt", [128, 2 * HEADS * 128], F32R,
                          kind="ExternalInput").ap()
    m_d = nc.dram_tensor("m", [128, HEADS * HID], F32R, kind="ExternalInput").ap()
    out_d = nc.dram_tensor("out", [NT, KH, 128, SQ], F32,
                           kind="ExternalOutput").ap()

    with tile.TileContext(nc) as tc:
        with ExitStack() as ctx:
            ctx.enter_context(nc.allow_low_precision(
                "bf16/f32r matmul inputs; accumulation stays f32 in PSUM"))
            const = ctx.enter_context(tc.tile_pool(name="const", bufs=1))
            wqm_t = []
            for m in range(KH):
                wqm_t.append(const.tile([128, HID], BF16, name=f"wqm{m}"))
            nc.sync.dma_start(wqm_t[0][:], wqm_d[0])
            kt_t = const.tile([128, 2 * HEADS * 128], F32R, name="kt_t")
            m_t = const.tile([128, HEADS * HID], F32R, name="m_t")

            def dma_const_early():
                nc.sync.dma_start(wqm_t[1][:], wqm_d[1])
                nc.sync.dma_start(kt_t[:], kt_d)

            def dma_const_rest():
                nc.sync.dma_start(m_t[:], m_d)
                for m in range(2, KH):
                    nc.sync.dma_start(wqm_t[m][:], wqm_d[m])

            xt_pool = ctx.enter_context(tc.tile_pool(name="xt", bufs=4))
            qt_pool = ctx.enter_context(tc.tile_pool(name="qt", bufs=2))
            psum_mm = ctx.enter_context(
                tc.tile_pool(name="psum_mm", bufs=2, space="PSUM"))

            xt_tiles = {}

            def prefetch_xt(t):
                xh = []
                for hf in range(2):
                    xx = xt_pool.tile([128, XH], BF16, tag="xt", name=f"xt{t}_{hf}")
                    nc.sync.dma_start(xx[:], xt_d[t, hf])
                    xh.append(xx)
                xt_tiles[t] = xh

            exp_pool = ctx.enter_context(tc.tile_pool(name="exp", bufs=8))

            if loop_reps > 1:
                dma_const_early()
                dma_const_rest()
                # hint_engines: the ~2000-inst body exceeds IRAM blocks, so
                # prefetch the back-edge target (else ~4us I$ miss/iteration
                # inflates the measured per-pass slope)
                ctx.enter_context(tc.For_i(
                    0, loop_reps, 1,
                    hint_engines=(mybir.EngineType.PE, mybir.EngineType.DVE,
                                  mybir.EngineType.Activation,
                                  mybir.EngineType.SP, mybir.EngineType.Pool)))
                prefetch_xt(0)
                prefetch_xt(1)
            else:
                prefetch_xt(0)
                dma_const_early()
                prefetch_xt(1)
                dma_const_rest()

            qt_tiles = {}

            def b_group_makers(t):
                """B-phase of tile t as closures: per m-group, two 5-matmul
                halves + ScalarE eviction; first call prefetches xt(t+1)."""
                qt_t = qt_pool.tile([128, KH * SQ], F32R, tag="qt", name=f"qt{t}")
                qt_tiles[t] = qt_t
                state = {}
                makers = []

                def mk(m, half):
                    def run():
                        if m == 0 and half == 0 and t + 1 < NT \
                                and t + 1 not in xt_tiles:
                            prefetch_xt(t + 1)
                        if m not in state:
                            state[m] = psum_mm.tile([128, SQ], F32, tag="mm",
                                                    name=f"pq{t}_{m}")
                        pq = state[m]
                        xh = xt_tiles[t]
                        for k in range(5 * half, 5 * half + 5):
                            nc.tensor.matmul(
                                pq[:],
                                wqm_t[m][:, k * 128 : (k + 1) * 128],
                                xh[k // 5][:, (k % 5) * SQ : (k % 5 + 1) * SQ],
                                start=(k == 0), stop=(k == KH - 1),
                            )
                        if half == 1:
                            nc.scalar.copy(qt_t[:, m * SQ : (m + 1) * SQ], pq[:])
                            if m == KH - 1:
                                xt_tiles.pop(t)
                    return run

                for m in range(KH):
                    makers.append(mk(m, 0))
                    makers.append(mk(m, 1))
                return makers

            den_pool = ctx.enter_context(tc.tile_pool(name="den", bufs=2))
            rd_pool = ctx.enter_context(tc.tile_pool(name="rd", bufs=2))
            rdb_pool = ctx.enter_context(tc.tile_pool(name="rdb", bufs=2))
            out_pool = ctx.enter_context(tc.tile_pool(name="outsb", bufs=3))
            psum_at = ctx.enter_context(
                tc.tile_pool(name="psum_at", bufs=2, space="PSUM"))
            psum_out = ctx.enter_context(
                tc.tile_pool(name="psum_out", bufs=4, space="PSUM"))

            def phase_C(t, fillers):
                """Attention + rank-77 output accumulation; `fillers` (B-phase
                groups of t+1) emitted between dependent links as PE gap
                fillers."""
                qt_t = qt_tiles.pop(t)
                sc_tiles, exp_tiles, rds = {}, {}, {}
                po = {}

                def fill(n=1):
                    for _ in range(n):
                        if fillers:
                            fillers.pop(0)()

                def out_mms(n_lo, n_cnt, h):
                    for n in range(n_lo, n_lo + n_cnt):
                        if n not in po:
                            po[n] = psum_out.tile([128, SQ], F32, tag="po",
                                                  name=f"po{t}_{n}")
                        nc.tensor.matmul(
                            po[n][:],
                            m_t[:, h * HID + n * 128 : h * HID + (n + 1) * 128],
                            exp_tiles[h][:],
                            start=(h == 0), stop=(h == HEADS - 1),
                        )
                        if h == HEADS - 1:
                            ob = out_pool.tile([128, SQ], F32, tag="ob",
                                               name=f"ob{t}_{n}")
                            nc.scalar.copy(ob[:], po.pop(n)[:])
                            nc.sync.dma_start(out_d[t, n], ob[:])

                def stage_sc(h):  # scoresT, both pieces at full K=128
                    sc = psum_at.tile([128, SQ], F32, tag="attn", name=f"sc{t}_{h}")
                    nc.tensor.matmul(
                        sc[:], kt_t[:, h * 128 : (h + 1) * 128],
                        qt_t[:, h * SQ : (h + 1) * SQ],
                        start=True, stop=False)
                    c2 = 8 + h // 4
                    nc.tensor.matmul(
                        sc[:], kt_t[:, (8 + h) * 128 : (9 + h) * 128],
                        qt_t[:, c2 * SQ : (c2 + 1) * SQ],
                        start=False, stop=True)
                    sc_tiles[h] = sc

                def stage_mid(h):  # exp, den on Pool, 1/den on DVE
                    sc = sc_tiles.pop(h)
                    exp_h = exp_pool.tile([128, SQ], F32R, tag="exp",
                                          name=f"ex{t}_{h}")
                    nc.scalar.activation(
                        exp_h[:], sc[:],
                        mybir.ActivationFunctionType.Exp, scale=ATTN_SCALE,
                    )
                    exp_tiles[h] = exp_h
                    den = den_pool.tile([SKEY, SQ], F32, tag="den",
                                        name=f"dn{t}_{h}")
                    nc.gpsimd.partition_all_reduce(
                        den[:], exp_h[0:SKEY, :], SKEY, ReduceOp.add)
                    rd = rd_pool.tile([1, SQ], F32, tag="rd", name=f"rd{t}_{h}")
                    nc.vector.reciprocal_approx_fast(rd[:], den[0:1, :])
                    rds[h] = rd

                def stage_out(h):  # broadcast 1/den, normalize, accumulate out
                    rdb = rdb_pool.tile([SKEY, SQ], F32, tag="rdb",
                                        name=f"rb{t}_{h}")
                    nc.gpsimd.partition_broadcast(rdb[:], rds.pop(h)[:], SKEY)
                    fill()
                    nc.vector.tensor_tensor(
                        exp_tiles[h][0:SKEY, :], exp_tiles[h][0:SKEY, :],
                        rdb[:], mybir.AluOpType.mult)
                    fill()
                    out_mms(PASSES[0][0], PASSES[0][1], h)

                for s in range(HEADS + 2):
                    if s < HEADS:
                        stage_sc(s)
                    fill()
                    if 0 <= s - 1 < HEADS:
                        stage_mid(s - 1)
                    fill()
                    if 0 <= s - 2 < HEADS:
                        stage_out(s - 2)
                for (n_lo, n_cnt) in PASSES[1:]:
                    for h in range(HEADS):
                        out_mms(n_lo, n_cnt, h)
                        fill()
                while fillers:
                    fillers.pop(0)()

            for run in b_group_makers(0):
                run()
            for t in range(NT):
                fillers = b_group_makers(t + 1) if t + 1 < NT else []
                phase_C(t, fillers)

    nc.finalize()
    return nc


from concourse.bass_utils import run_bass_kernel_spmd

_NC_CACHE = {}


def _get_nc(loop_reps=1):
    if loop_reps not in _NC_CACHE:
        _NC_CACHE[loop_reps] = build_nc(loop_reps)
    return _NC_CACHE[loop_reps]


def kernel(**inputs):
    inputs = {k: np.asarray(v) for k, v in inputs.items()}
    wq, wk, wv, wo = fold_weights(inputs)
    x = inputs["hidden_states"].astype(np.float32, copy=False)
    enc = inputs["encoder_hidden_states"].astype(np.float64, copy=False)
    B = x.shape[0]
    in_maps = [make_in_map(x[b], enc[b], wq, wk, wv, wo) for b in range(B)]
    nc = _get_nc()
    res = run_bass_kernel_spmd(nc, in_maps, list(range(B)))
    bout = inputs["bout"].astype(np.float32, copy=False)
    outs = []
    for b in range(B):
        ot = res.results[b]["out"]              # [NT, KH, 128, SQ]
        o = ot.transpose(0, 3, 1, 2).reshape(SEQ, HID)
        outs.append(o + bout[None, :])
    return np.stack(outs)
